# revision 38
# baseline (speedup 1.0000x reference)
"""Trainium2 Bass kernel for nn_KeyRecorder.

Math (reference):
  comp = LN(relu(obs @ W1 + b1)) * g1 + bl1          [B, T, R]
  past = max(comp[:, :-20:10, :], axis=time)          408 strided rows
  gmax = max(cummax(comp[:, -20:, :]), past)          [B, 20, R]
  out  = LN(relu(gmax @ W2 + b2)) * g2 + bl2          [B, 20, D]

Only 428 of the 4096 timesteps per batch element are consumed (408
strided + last 20); the host gathers those rows, transposes to d-major,
casts to fp16 and ships ~0.9 MB/core.  Batch sharded 2-per-core across
8 cores, no collectives.

LN1's affine (g1, bl1) is folded into W2/b2 on the host (valid since
g1 >= 0, asserted); when ln2 is the identity affine the final scale/
shift is compiled out entirely.  b1/b2 bias adds are folded into the
matmuls as rank-1 updates.  LN1 runs in token-partition layout with all
7 token tiles' stats batched into [128, 7] ops; E[x^2]-mu^2 variance
avoids the centering pass.  A dummy Sqrt primes the Scalar engine's
act-table (sqrt set covers relu/square/identity too) off the critical
path; dead matmuls keep the PE out of its low p-state while the input
DMA is in flight.  GpSimd is never used for compute (per-op dispatch
costs ~1-4us there).
"""

import os
import numpy as np

import concourse.bass as bass
import concourse.bacc as bacc
import concourse.mybir as mybir
import concourse.tile as tile
from concourse.bass_utils import run_bass_kernel_spmd

F32 = mybir.dt.float32
F16 = mybir.dt.float16
ALU = mybir.AluOpType
ACT = mybir.ActivationFunctionType
AX = mybir.AxisListType

B, T, D, R = 16, 4096, 512, 64
LOCAL, SR, EPS = 20, 10, 1e-5
N_CORES = 8
BPC = B // N_CORES            # batch elements per core
NSTR = (T - LOCAL + SR - 1) // SR   # 408 strided past rows
NSEL = NSTR + LOCAL           # 428 rows consumed per batch element
GRP = 448                     # per-batch group width in SBUF (428 padded)
NTOK = GRP * BPC              # 896 token columns per core
NTT = NTOK // 128             # 7 token tiles
DC = D // 128                 # 4 contraction chunks
NO = BPC * LOCAL              # 40 output rows per core
CA = 512                      # token cols in group A
CB = NTOK - CA                # 384 token cols in group B
PAD = 16                      # -big pad for the copy-free cummax
NEG = -60000.0                # finite fp16 "-inf"
WARM = 10                     # PE warm-up matmuls during the DMA wait

IDX = np.array(list(range(0, T - LOCAL, SR)) + list(range(T - LOCAL, T)))

_cache: dict = {}


def _build_program(ln2_affine: bool):
    """Build + compile the per-core Bass program once per variant."""
    key = ("nc", ln2_affine)
    if key in _cache:
        return _cache[key]

    nc = bacc.Bacc("TRN2", target_bir_lowering=False, debug=False,
                   enable_asserts=False)

    # cpack: w1 [:, 0:256] ([128, 4, 64] chunk-major), w2 rows 0:64 at
    # [*, 256:768], identity at [:, 768:896]
    cpack_d = nc.dram_tensor("cpack", [128, 896], F16, kind="ExternalInput")
    # rowc: b1 cols 0:64, b2 cols 64:576
    rowc_d = nc.dram_tensor("rowc", [1, 576], F16, kind="ExternalInput")
    obsA_d = nc.dram_tensor("obsA", [128, DC, CA], F16, kind="ExternalInput")
    obsB_d = nc.dram_tensor("obsB", [128, DC, CB], F16, kind="ExternalInput")
    if ln2_affine:
        g2bl2_d = nc.dram_tensor("g2bl2", [NO, 2 * D], F32,
                                 kind="ExternalInput")
    out_d = nc.dram_tensor("out16", [NO, D], F16, kind="ExternalOutput")

    inv_r = 1.0 / R
    inv_d = 1.0 / D
    H = D // 2

    with tile.TileContext(nc) as tc:
        with (
            tc.tile_pool(name="const", bufs=1) as cpool,
            tc.tile_pool(name="ps", bufs=1, space=bass.MemorySpace.PSUM) as pp,
        ):
            # ---- input DMAs 3-wide: gpsimd SWDGE (fast desc-gen) takes the
            # big obsA stream while both HWDGE queues carry the rest ----
            obsA = cpool.tile([128, DC, CA], F16)
            nc.gpsimd.dma_start(obsA[:], obsA_d[:])
            obsB = cpool.tile([128, DC, CB], F16)
            nc.sync.dma_start(obsB[:], obsB_d[:])
            cpack = cpool.tile([128, 896], F16)
            nc.scalar.dma_start(cpack[:], cpack_d[:])
            rowc = cpool.tile([1, 576], F16)
            nc.scalar.dma_start(rowc[:], rowc_d[:])
            if ln2_affine:
                g2bl2 = cpool.tile([NO, 2 * D], F32)
                nc.gpsimd.dma_start(g2bl2[:], g2bl2_d[:])

            warm16 = cpool.tile([128, 128], F16)
            nc.vector.memset(warm16[:], 0.5)
            ones16 = cpool.tile([1, CA], F16)
            nc.vector.memset(ones16[:], 1.0)
            identv = cpack[:, 768:896]

            # ---- PE p-state warm-up: dead matmuls while DMA lands ----
            pwarm = pp.tile([128, 128], F32)
            for _ in range(WARM):
                nc.tensor.matmul(pwarm[:], warm16[:], warm16[:],
                                 start=True, stop=True)

            # ---- stage 1 matmuls: pre[r, t] = W1^T obs + b1 (rank-1) ----
            pgA = pp.tile([R, CA], F32)
            for c in range(DC):
                nc.tensor.matmul(pgA[:], cpack[:, c * 64:(c + 1) * 64],
                                 obsA[:, c, :], start=(c == 0), stop=False)
            nc.tensor.matmul(pgA[:], rowc[0:1, 0:R], ones16[0:1, 0:CA],
                             start=False, stop=True)
            pgB = pp.tile([R, CB], F32)
            for c in range(DC):
                nc.tensor.matmul(pgB[:], cpack[:, c * 64:(c + 1) * 64],
                                 obsB[:, c, :], start=(c == 0), stop=False)
            nc.tensor.matmul(pgB[:], rowc[0:1, 0:R], ones16[0:1, 0:CB],
                             start=False, stop=True)

            # ---- relu into fp16 [r, t], split DVE / Scalar ----
            xr = cpool.tile([R, NTOK], F16)
            nc.vector.tensor_scalar(xr[:, 0:256], pgA[:, 0:256], 0.0, None,
                                    ALU.max)
            nc.scalar.activation(xr[:, 256:512], pgA[:, 256:512], ACT.Relu)
            nc.vector.tensor_scalar(xr[:, 512:768], pgB[:, 0:256], 0.0, None,
                                    ALU.max)
            nc.scalar.activation(xr[:, 768:896], pgB[:, 256:384], ACT.Relu)

            # ---- transpose to token-partition layout [t, tile, r] ----
            xrT = pp.tile([128, NTT, R], F16)
            for tt in range(NTT):
                nc.tensor.transpose(xrT[:, tt, :], xr[:, bass.ts(tt, 128)],
                                    identv[0:R, 0:R])
            # dead matmuls bridge the PE through the stats/affine window
            for _ in range(12):
                nc.tensor.matmul(pwarm[:], warm16[:], warm16[:],
                                 start=True, stop=True)

            # ---- batched LN1 stats across all 7 tiles: [128, 7] ----
            # DVE owns the serial chain; Scalar computes the squares in
            # parallel (same act table as Sqrt, loaded once at the top)
            sq16 = cpool.tile([128, NTT, R], F16)
            nc.scalar.activation(sq16[:], xrT[:], ACT.Square)
            rsum = cpool.tile([128, NTT], F32)
            nc.vector.reduce_sum(rsum[:], xrT[:], axis=AX.X)
            ssq = cpool.tile([128, NTT], F32)
            nc.vector.reduce_sum(ssq[:], sq16[:], axis=AX.X)
            negmu = cpool.tile([128, NTT], F32)
            nc.vector.tensor_scalar_mul(negmu[:], rsum[:], -inv_r)
            musq = cpool.tile([128, NTT], F32)
            nc.vector.tensor_tensor(musq[:], negmu[:], negmu[:], op=ALU.mult)
            vs = cpool.tile([128, NTT], F32)
            nc.vector.tensor_scalar(vs[:], ssq[:], inv_r, EPS, ALU.mult,
                                    ALU.add)
            var = cpool.tile([128, NTT], F32)
            nc.vector.tensor_tensor(var[:], vs[:], musq[:], op=ALU.subtract)
            std = cpool.tile([128, NTT], F32)
            nc.scalar.activation(std[:], var[:], ACT.Sqrt)
            rstd = cpool.tile([128, NTT], F32)
            nc.vector.reciprocal(rstd[:], std[:])
            # nmr = negmu * rstd feeds the Scalar-engine affine variant
            nmr = cpool.tile([128, NTT], F32)
            nc.vector.tensor_tensor(nmr[:], negmu[:], rstd[:], op=ALU.mult)

            # ---- per-tile affine (x - mu) * rstd, DVE/Scalar alternating,
            # tiles 3 and 6 first so the cummax region lands early ----
            TORD = (3, 6, 0, 1, 2, 4, 5)
            y16 = cpool.tile([128, NTT, R], F16)
            for i, tt in enumerate(TORD):
                if i % 2 == 0:
                    nc.vector.tensor_scalar(y16[:, tt, :], xrT[:, tt, :],
                                            negmu[:, tt:tt + 1],
                                            rstd[:, tt:tt + 1],
                                            ALU.add, ALU.mult)
                else:
                    nc.scalar.activation(y16[:, tt, :], xrT[:, tt, :],
                                         ACT.Identity,
                                         bias=nmr[:, tt:tt + 1],
                                         scale=rstd[:, tt:tt + 1])

            # stage-2 tiles: cummax pads/copies start as soon as the
            # back-transpose of tiles 3 and 6 lands; past maxes fire per
            # batch once their column range is complete
            pa = cpool.tile([R, BPC, PAD + LOCAL], F16)
            pb = cpool.tile([R, BPC, PAD + LOCAL], F16)
            nc.vector.memset(pa[:, :, 0:PAD], NEG)
            nc.vector.memset(pb[:, :, 0:PAD], NEG)
            past01 = cpool.tile([R, BPC], F32)

            compT = pp.tile([R, NTOK], F16)
            for i, tt in enumerate(TORD):
                nc.tensor.transpose(compT[:, bass.ts(tt, 128)], y16[:, tt, :],
                                    identv[:])
                if tt == 3:       # batch-0 window cols 408:428 ready
                    nc.vector.tensor_copy(pa[:, 0, PAD:],
                                          compT[:, NSTR:NSEL])
                elif tt == 6:     # batch-1 window cols 856:876 ready
                    nc.scalar.activation(pa[:, 1, PAD:],
                                         compT[:, GRP + NSTR:GRP + NSEL],
                                         ACT.Copy)
                elif tt == 2:     # tiles {0..3} done: batch-0 past ready
                    nc.vector.reduce_max(past01[:, 0:1], compT[:, 0:NSTR],
                                         axis=AX.X)
                elif tt == 5:     # tiles {3..6} done: batch-1 past ready
                    nc.vector.reduce_max(past01[:, 1:2],
                                         compT[:, GRP:GRP + NSTR], axis=AX.X)

            # cummax over the last LOCAL frames; -inf pad kills the
            # prefix-copy of the log-step scan
            cur, nxt = pa, pb
            s = 1
            while s < LOCAL:
                nc.vector.tensor_tensor(nxt[:, :, PAD:], cur[:, :, PAD:],
                                        cur[:, :, PAD - s:PAD + LOCAL - s],
                                        op=ALU.max)
                cur, nxt = nxt, cur
                s *= 2

            gmax16 = cpool.tile([R, NO], F16)
            nc.vector.tensor_scalar(gmax16[:, 0:LOCAL], cur[:, 0, PAD:],
                                    past01[:, 0:1], None, ALU.max)
            nc.vector.tensor_scalar(gmax16[:, LOCAL:NO], cur[:, 1, PAD:],
                                    past01[:, 1:2], None, ALU.max)

            # keep the PE clocked through the cummax window
            for _ in range(12):
                nc.tensor.matmul(pwarm[:], warm16[:], warm16[:],
                                 start=True, stop=True)

            # ---- stage 3: out = LN(relu(gmax @ W2' + b2')) [* g2 + bl2] ----
            # bias rank-1 first: it has no dependence on gmax, so the PE
            # retires it while the cummax chain still runs
            ps2 = pp.tile([NO, D], F32)
            nc.tensor.matmul(ps2[:], ones16[0:1, 0:NO], rowc[0:1, 64:576],
                             start=True, stop=False)
            nc.tensor.matmul(ps2[:], gmax16[:], cpack[0:R, 256:768],
                             start=False, stop=True)

            xr2 = cpool.tile([NO, D], F32)
            rs2ab = cpool.tile([NO, 2], F32)
            nc.vector.tensor_scalar(xr2[:, 0:H], ps2[:, 0:H], 0.0, 0.0,
                                    ALU.max, ALU.add,
                                    accum_out=rs2ab[:, 0:1])
            nc.scalar.activation(xr2[:, H:D], ps2[:, H:D], ACT.Relu,
                                 accum_out=rs2ab[:, 1:2])
            rsum2 = cpool.tile([NO, 1], F32)
            nc.vector.tensor_reduce(rsum2[:], rs2ab[:], axis=AX.X,
                                    op=ALU.add)
            sq2 = cpool.tile([NO, D], F32)
            ssq2 = cpool.tile([NO, 1], F32)
            nc.scalar.activation(sq2[:], xr2[:], ACT.Square,
                                 accum_out=ssq2[:])
            negmu2 = cpool.tile([NO, 1], F32)
            nc.vector.tensor_scalar_mul(negmu2[:], rsum2[:], -inv_d)
            musq2 = cpool.tile([NO, 1], F32)
            nc.vector.tensor_tensor(musq2[:], negmu2[:], negmu2[:],
                                    op=ALU.mult)
            vs2 = cpool.tile([NO, 1], F32)
            nc.vector.tensor_scalar(vs2[:], ssq2[:], inv_d, EPS, ALU.mult,
                                    ALU.add)
            var2 = cpool.tile([NO, 1], F32)
            nc.vector.tensor_tensor(var2[:], vs2[:], musq2[:],
                                    op=ALU.subtract)
            std2 = cpool.tile([NO, 1], F32)
            nc.scalar.activation(std2[:], var2[:], ACT.Sqrt)
            rstd2 = cpool.tile([NO, 1], F32)
            nc.vector.reciprocal(rstd2[:], std2[:])
            nmr2 = cpool.tile([NO, 1], F32)
            nc.vector.tensor_tensor(nmr2[:], negmu2[:], rstd2[:], op=ALU.mult)

            if ln2_affine:
                y2 = cpool.tile([NO, D], F32)
                nc.vector.tensor_scalar(y2[:], xr2[:], negmu2[:], rstd2[:],
                                        ALU.add, ALU.mult)
                yg = cpool.tile([NO, D], F32)
                nc.vector.tensor_tensor(yg[:], y2[:], g2bl2[:, 0:D],
                                        op=ALU.mult)
                out_sb = cpool.tile([NO, D], F16)
                nc.vector.tensor_tensor(out_sb[:], yg[:], g2bl2[:, D:2 * D],
                                        op=ALU.add)
            else:
                out_sb = cpool.tile([NO, D], F16)
                nc.vector.tensor_scalar(out_sb[:, 0:H], xr2[:, 0:H],
                                        negmu2[:], rstd2[:],
                                        ALU.add, ALU.mult)
                nc.scalar.activation(out_sb[:, H:D], xr2[:, H:D],
                                     ACT.Identity, bias=nmr2[:],
                                     scale=rstd2[:])

            nc.sync.dma_start(out_d[:], out_sb[:])

    nc.compile()
    _cache[key] = nc
    return nc


def _host_inputs(obs, W1, b1, ln1_g, ln1_b, W2, b2, ln2_g, ln2_b,
                 ln2_affine):
    obs = np.ascontiguousarray(np.asarray(obs, dtype=np.float32))
    W1 = np.asarray(W1, np.float32)
    b1 = np.asarray(b1, np.float32)
    ln1_g = np.asarray(ln1_g, np.float32)
    ln1_b = np.asarray(ln1_b, np.float32)
    W2 = np.asarray(W2, np.float32)
    b2 = np.asarray(b2, np.float32)
    ln2_g = np.asarray(ln2_g, np.float32)
    ln2_b = np.asarray(ln2_b, np.float32)

    # folding LN1's affine past the max/cummax requires monotonicity
    assert np.all(ln1_g >= 0), "ln1_g must be >= 0 for the affine fold"

    cpack = np.zeros((128, 896), np.float16)
    cpack[:, 0:256] = W1.reshape(DC, 128, R).transpose(1, 0, 2).reshape(
        128, DC * R)
    cpack[0:R, 256:768] = (ln1_g[:, None] * W2).astype(np.float16)
    cpack[:, 768:896] = np.eye(128, dtype=np.float16)

    rowc = np.zeros((1, 576), np.float16)
    rowc[0, 0:R] = b1.astype(np.float16)
    rowc[0, R:R + D] = (b2 + ln1_b @ W2).astype(np.float16)

    shared = {"cpack": cpack, "rowc": rowc}
    if ln2_affine:
        g2bl2 = np.zeros((NO, 2 * D), np.float32)
        g2bl2[:, 0:D] = ln2_g
        g2bl2[:, D:2 * D] = ln2_b
        shared["g2bl2"] = g2bl2

    in_maps = []
    for c in range(N_CORES):
        sel = obs[BPC * c:BPC * (c + 1)][:, IDX, :]        # [BPC, 428, 512]
        grp = np.zeros((BPC, GRP, D), np.float16)
        grp[:, :NSEL] = sel
        obsT = grp.reshape(NTOK, D).T                      # [512, 896] fp16
        arr4 = obsT.reshape(DC, 128, NTOK).transpose(1, 0, 2)  # [128, 4, 896]
        obsA = np.ascontiguousarray(arr4[:, :, 0:CA])
        obsB = np.ascontiguousarray(arr4[:, :, CA:NTOK])
        in_maps.append({"obsA": obsA, "obsB": obsB, **shared})
    return in_maps


def _install_ntff_shim():
    """The agent image's antenv lacks axon_hooks; synthesize it so
    trace=True can reach the libaxon NTFF profiler (test-time only)."""
    import sys
    import types
    if "antenv.axon_hooks" in sys.modules:
        return True
    try:
        import antenv
        from trn_agent_boot.trn_boot import _ntff_profile_via_ctypes
    except ImportError:
        return False
    so_path = "/opt/axon/libaxon_pjrt.so"
    if not os.path.exists(so_path):
        return False
    hook = _ntff_profile_via_ctypes(so_path)
    mod = types.ModuleType("antenv.axon_hooks")
    mod._hook = hook
    mod.set_axon_ntff_profile_hook = lambda h: setattr(mod, "_hook", h)
    mod.get_axon_ntff_profile_hook = lambda: mod._hook
    sys.modules["antenv.axon_hooks"] = mod
    antenv.axon_hooks = mod
    return hook is not None


def kernel(obs_frames, W1, b1, ln1_g, ln1_b, W2, b2, ln2_g, ln2_b):
    ln2_affine = not (np.allclose(np.asarray(ln2_g), 1.0)
                      and np.allclose(np.asarray(ln2_b), 0.0))
    nc = _build_program(ln2_affine)
    in_maps = _host_inputs(obs_frames, W1, b1, ln1_g, ln1_b,
                           W2, b2, ln2_g, ln2_b, ln2_affine)
    trace = bool(os.environ.get("BASS_TRACE"))
    if trace:
        trace = _install_ntff_shim()
        import concourse.bass_utils as _bu
        _bu.upload_artifacts = lambda tmpdir: f"local://{tmpdir}"
    res = run_bass_kernel_spmd(nc, in_maps, core_ids=list(range(N_CORES)),
                               trace=trace)
    _cache["last_result"] = res
    out = np.stack([res.results[c]["out16"].astype(np.float32)
                    .reshape(BPC, LOCAL, D) for c in range(N_CORES)])
    return out.reshape(B, LOCAL, D)
